# revision 27
# baseline (speedup 1.0000x reference)
"""Masked 5x5 conv (PixelCNN 'A' mask) on 8 Trainium2 NeuronCores.

Problem (hardcoded): x[4,192,128,128] f32, weight[384,192,5,5] f32,
bias[384] f32, mask[4,1,128,128] i32.
out = where(window_any(mask), conv(x, weight*maskA) + bias, 0).

The 'A' causal mask keeps 12 of 25 taps: rows kh=0,1 fully, row kh=2 only
kw=0,1 -- i.e. every tap reads the current output row or rows above it.

Sharding: core c = (batch b = c//2, row-half = c%2). Each core computes one
batch's 64 output rows for all 384 out channels (3 M=128 chunks).

Per output tile [128 cout, 4 rows x 128 cols = 512] we accumulate 16
matmuls into one PSUM bank:
  - 2 fp8 DoubleRow matmuls (K=256): tap pairs (0,0)+(1,0) and (0,2)+(1,2)
    on ci[0:128], operands e4m3 (x/8 and 8*w so products keep natural
    scale).  DoubleRow streams the two K-groups as dim-1 of a [128,2,...]
    AP; group 1 reads a row-shifted fp8 copy so the pair step is a fixed
    whole-tensor offset (16-byte aligned).
  - 8 bf16 K=128 matmuls for the remaining ci[0:128] taps (from tile xa)
  - 5 bf16 tap-PAIRS x ci[128:192]      (tile xb: lower 64 partitions =
    ci[128:192] data, upper 64 = same data shifted 1 col, so one K=128
    matmul covers two taps that differ by (0,+1))
  - 1 bf16 tap-pair (0,4)+(1,4) x ci[128:192] (tile xc: upper shifted +1 row)
4 of 18 K-slabs in fp8 keeps rel err ~0.017 (<2e-2) while cutting the PE
stream ~10%.
Epilogue: one DVE scalar_tensor_tensor: out = (psum + bias) * valid.
"""

import numpy as np
import ml_dtypes

import concourse.bass as bass
import concourse.tile as tile
from concourse import mybir
from concourse.bass_utils import run_bass_kernel_spmd

B, CIN, COUT, H, W = 4, 192, 384, 128, 128
KH = KW = 5
PAD = 2
NCORES = 8
HHALF = 64          # output rows per core
NROWS = HHALF + 2   # input rows staged per core (2 above)
WP = W + 4          # padded width (bf16 tensors)
FLAT = NROWS * WP   # 66*132 = 8712
WP8 = 144           # fp8 row pitch (16B-aligned so DoubleRow step%16==0)
FLAT8 = NROWS * WP8  # 66*144 = 9504
RB = 4              # output rows per block
NBLK = HHALF // RB  # 16 blocks
NFREE = RB * W      # 512 = one PSUM bank of fp32
NT = 3 * NBLK       # 48 tiles

# bf16 xa taps (ci 0:128) -- the 8 'A'-mask taps not covered by fp8 pairs
TAPS_BF = [(0, 1), (0, 3), (0, 4),
           (1, 1), (1, 3), (1, 4),
           (2, 0), (2, 1)]
# fp8 DoubleRow pairs: taps (0,kw)+(1,kw) on ci[0:128]
DR_KW = [0, 2]
# ci[128:192] handled as bf16 pairs packed into K=128 matmuls.
# slab xb (upper shifted +1 element = +1 col): pairs differing by (0,1)
PAIRS_XB = [((0, 0), (0, 1)), ((0, 2), (0, 3)),
            ((1, 0), (1, 1)), ((1, 2), (1, 3)), ((2, 0), (2, 1))]
# slab xc (upper shifted +132 elements = +1 row): the leftover pair
PAIR_XC = ((0, 4), (1, 4))

NSLOT = len(TAPS_BF) + len(PAIRS_XB) + 1   # 14 bf16 weight slots
SBC0 = len(TAPS_BF)                        # first xb/xc slot index

BF16 = ml_dtypes.bfloat16
F8 = ml_dtypes.float8_e4m3


def _build_program():
    """Raw Bass (no Tile): this walrus build rejects instructions carrying
    more than ~1 embedded sync wait, so all synchronization is standalone
    wait_ge instructions with manually-managed semaphores.

    Schedule (per core):
      - PE pre-warm: dummy matmuls during the initial DMA wait flip the
        HAM clock gate toward 2.4 GHz before the real stream begins.
      - Input DMAs stream in prioritized FIFO waves (queues are ~45-90
        GB/s each, ~358 GB/s aggregate); wave-1a is kept tiny (m=0
        weights for the first slots + first rows of xa/x8) so real
        matmuls start ~10us in.
      - Phase A runs the 2 DR + 8 xa slots of tiles 0..7 as the first
        x rows land; phase B completes those tiles with the xb/xc pair
        slots; then steady state: 16 matmuls per [128 x 512] PSUM tile.
      - DVE fuses (psum + bias) * valid into one scalar_tensor_tensor per
        tile, writing a bf16 staging buffer; outputs stream out in 2-tile
        chunks with a tapered, split final chunk."""
    nc = bass.Bass()
    bf = mybir.dt.bfloat16
    f8 = mybir.dt.float8e4
    f32 = mybir.dt.float32

    xa_d = nc.dram_tensor("xa", [128, FLAT], bf, kind="ExternalInput")
    x8_d = nc.dram_tensor("x8", [128, FLAT8], f8, kind="ExternalInput")
    xb_d = nc.dram_tensor("xb", [128, FLAT], bf, kind="ExternalInput")
    xc_d = nc.dram_tensor("xc", [128, FLAT], bf, kind="ExternalInput")
    wt_d = nc.dram_tensor("wt", [128, 3 * NSLOT * 128], bf, kind="ExternalInput")
    wdr_d = nc.dram_tensor("wdr", [128, 3 * 2 * 256], f8, kind="ExternalInput")
    bt_d = nc.dram_tensor("bt", [128, 3], f32, kind="ExternalInput")
    vt_d = nc.dram_tensor("vt", [128, HHALF * W], bf, kind="ExternalInput")
    out_d = nc.dram_tensor("out", [128, 3 * HHALF * W], bf, kind="ExternalOutput")

    NPS = 8           # psum banks in rotation
    PHA = 8           # tiles 0..PHA-1 run split-phase (xa/DR first, xb/xc later)
    OCH = 2           # out-DMA granularity: blocks per chunk
    NDUMMY = 11       # PE pre-warm matmuls (sized to end ~at the clock flip)
    DR = mybir.MatmulPerfMode.DoubleRow

    # row boundaries for the input waves (staged row index)
    R0 = 7            # wave 0: xa rows for tile 0
    R08 = 5           # wave 0: x8 rows for tile 0
    R1A = 14          # wave 1a: rows ..13  (tiles 1..2)
    R1B = 38          # wave 1b: rows 14..37 (tiles 3..7 + phase B)
    RG1 = 52          # wave 3 group 1: rows 38..51 (tiles 8..11)

    from contextlib import ExitStack
    with ExitStack() as ctx:
        xa_t = ctx.enter_context(nc.sbuf_tensor([128, FLAT], bf))
        x8_t = ctx.enter_context(nc.sbuf_tensor([128, FLAT8], f8))
        xb_t = ctx.enter_context(nc.sbuf_tensor([128, FLAT], bf))
        xc_t = ctx.enter_context(nc.sbuf_tensor([128, FLAT], bf))
        wt_t = ctx.enter_context(nc.sbuf_tensor([128, 3 * NSLOT * 128], bf))
        wdr_t = ctx.enter_context(nc.sbuf_tensor([128, 3 * 2 * 256], f8))
        bt_t = ctx.enter_context(nc.sbuf_tensor([128, 3], f32))
        vt_t = ctx.enter_context(nc.sbuf_tensor([128, HHALF * W], bf))
        st_t = ctx.enter_context(nc.sbuf_tensor([128, 3 * HHALF * W], bf))
        ps_t = ctx.enter_context(nc.psum_tensor([128, NPS * NFREE], f32))
        d0a1 = ctx.enter_context(nc.semaphore("d0a1"))
        d0a2 = ctx.enter_context(nc.semaphore("d0a2"))
        d0b = ctx.enter_context(nc.semaphore("d0b"))
        d1a = ctx.enter_context(nc.semaphore("d1a"))
        d1b = ctx.enter_context(nc.semaphore("d1b"))
        dbc = ctx.enter_context(nc.semaphore("dbc"))    # wt sBC m0 + xb/xc rows<38
        dvt1a = ctx.enter_context(nc.semaphore("dvt1a"))  # bt + vt blks 0-1
        dvt1b = ctx.enter_context(nc.semaphore("dvt1b"))  # vt blks 2-7
        dvt2 = ctx.enter_context(nc.semaphore("dvt2"))  # vt second half
        dg1 = ctx.enter_context(nc.semaphore("dg1"))    # x rows 38..51
        dg2 = ctx.enter_context(nc.semaphore("dg2"))    # x rows 52..65
        dw2 = ctx.enter_context(nc.semaphore("dw2"))    # wt/wdr m1,m2
        pes = ctx.enter_context(nc.semaphore("pes"))
        dve = ctx.enter_context(nc.semaphore("dve"))
        dout = ctx.enter_context(nc.semaphore("dout"))
        block = ctx.enter_context(nc.Block())
        xa_v = xa_t[:].rearrange("p (r c) -> p r c", c=WP)
        xb_v = xb_t[:].rearrange("p (r c) -> p r c", c=WP)
        xc_v = xc_t[:].rearrange("p (r c) -> p r c", c=WP)
        # DoubleRow rhs [p, two, r, c]: dim 1 (the K-group pair) strides by
        # one fp8 row (144B, 16B-aligned), overlapping the row dim -- group
        # 0 reads rows j0.. (kh=0) and group 1 rows j0+1.. (kh=1).
        from concourse.ap import AP as _AP
        _x8ap = x8_t[:]

        def x8_dr(j0, kw, rb):
            return _AP(_x8ap.tensor, j0 * WP8 + kw,
                       [[FLAT8, 128], [WP8, 2], [WP8, rb], [1, W]])

        # bf16 slots: (weight-slot index, view, kh, kw)
        slots_a = [(s, xa_v, kh, kw) for s, (kh, kw) in enumerate(TAPS_BF)]
        slots_bc = [(SBC0 + i, xb_v, ta[0], ta[1])
                    for i, (ta, _tb) in enumerate(PAIRS_XB)]
        slots_bc += [(SBC0 + 5, xc_v, PAIR_XC[0][0], PAIR_XC[0][1])]

        def wt_ap(m, s):
            o = (m * NSLOT + s) * 128
            return wt_t[:, o:o + 128]

        def wdr_ap(m, pr):
            o = (m * 2 + pr) * 256
            return wdr_t[:, o:o + 256].rearrange("p (two m1) -> p two m1", two=2)

        def emit_phase_a(tensor, k, start):
            """2 DR + 8 bf16 xa matmuls of tile k (no stop)."""
            m, blk = divmod(k, NBLK)
            j0 = blk * RB
            ps = ps_t[:, (k % NPS) * NFREE:(k % NPS + 1) * NFREE]
            for pr, kw in enumerate(DR_KW):
                nc.tensor.matmul(
                    ps, wdr_ap(m, pr),
                    x8_dr(j0, kw, RB),
                    start=(start and pr == 0), stop=False, perf_mode=DR,
                )
            for s, view, kh, kw in slots_a:
                nc.tensor.matmul(
                    ps, wt_ap(m, s),
                    view[:, j0 + kh: j0 + kh + RB, kw: kw + W],
                    start=False, stop=False,
                )

        def emit_phase_b(tensor, k):
            """6 bf16 xb/xc pair matmuls of tile k (stop on last)."""
            m, blk = divmod(k, NBLK)
            j0 = blk * RB
            ps = ps_t[:, (k % NPS) * NFREE:(k % NPS + 1) * NFREE]
            n = len(slots_bc)
            for i, (s, view, kh, kw) in enumerate(slots_bc):
                mm = nc.tensor.matmul(
                    ps, wt_ap(m, s),
                    view[:, j0 + kh: j0 + kh + RB, kw: kw + W],
                    start=False, stop=(i == n - 1),
                )
                if i == n - 1:
                    mm.then_inc(pes, 1)

        @block.scalar
        def _(scalar):
            # the very last output quarter, issued in parallel with sync's
            scalar.wait_ge(dve, NT + 1)
            a = (NT - 1) * NFREE + 3 * (NFREE // 4)
            scalar.dma_start(out_d[:, a:a + NFREE // 4],
                             st_t[:, a:a + NFREE // 4]).then_inc(dout, 16)

        @block.sync
        def _(sync):
            # Queues drain FIFO, so pure issue order gives wave priority.
            # Each dma_start costs ~650ns of issue time on this engine, so
            # the head waves use few, need-ordered descriptors.
            def splitn(dst, src, lo, hi, sem, n):
                step = ((hi - lo) // n // 16) * 16
                for i in range(n):
                    a = lo + i * step
                    b = hi if i == n - 1 else a + step
                    sync.dma_start(dst[:, a:b], src[:, a:b]).then_inc(sem, 16)

            # wave 0: exactly what tile 0 needs, gated at slot-group
            # granularity so its matmuls start as pieces land
            splitn(xa_t, xa_d, 0, 5 * WP, d0a1, 1)
            splitn(wt_t, wt_d, 0, 4 * 128, d0a1, 1)
            splitn(xa_t, xa_d, 5 * WP, R0 * WP, d0a2, 1)
            splitn(wt_t, wt_d, 4 * 128, SBC0 * 128, d0a2, 1)
            splitn(x8_t, x8_d, 0, R08 * WP8, d0b, 1)
            sync.dma_start(wdr_t[:, 0:512], wdr_d[:, 0:512]).then_inc(d0b, 16)
            # wave 1a: x rows out to 13 (tiles 1..2)
            splitn(xa_t, xa_d, R0 * WP, R1A * WP, d1a, 1)
            splitn(x8_t, x8_d, R08 * WP8, R1A * WP8, d1a, 1)
            # wave 1b: x rows 14..37 for tiles 3..7  (6 DMAs)
            splitn(xa_t, xa_d, R1A * WP, R1B * WP, d1b, 2)
            splitn(x8_t, x8_d, R1A * WP8, R1B * WP8, d1b, 2)
            # wave 2: phase-B inputs first, then the valid/bias epilogue
            # inputs (vt in a small leading chunk so the first stt can run)
            sync.dma_start(bt_t[:], bt_d[:]).then_inc(dvt1a, 16)
            sync.dma_start(wt_t[:, SBC0 * 128:NSLOT * 128],
                           wt_d[:, SBC0 * 128:NSLOT * 128]).then_inc(dbc, 16)
            splitn(xb_t, xb_d, 0, R1B * WP, dbc, 3)
            splitn(xc_t, xc_d, 0, R1B * WP, dbc, 3)
            splitn(vt_t, vt_d, 0, 2 * RB * W, dvt1a, 1)
            splitn(vt_t, vt_d, 2 * RB * W, (HHALF // 2) * W, dvt1b, 3)
            # wave 3 group 1: all x, rows 38..51 (tiles 8..11)
            splitn(xa_t, xa_d, R1B * WP, RG1 * WP, dg1, 2)
            splitn(x8_t, x8_d, R1B * WP8, RG1 * WP8, dg1, 1)
            splitn(xb_t, xb_d, R1B * WP, RG1 * WP, dg1, 2)
            splitn(xc_t, xc_d, R1B * WP, RG1 * WP, dg1, 2)
            # vt second half (DVE needs it from tile 8)
            splitn(vt_t, vt_d, (HHALF // 2) * W, HHALF * W, dvt2, 3)
            # wave 3 group 2: all x, rows 52..65 (tiles 12..15)
            splitn(xa_t, xa_d, RG1 * WP, FLAT, dg2, 2)
            splitn(x8_t, x8_d, RG1 * WP8, FLAT8, dg2, 1)
            splitn(xb_t, xb_d, RG1 * WP, FLAT, dg2, 2)
            splitn(xc_t, xc_d, RG1 * WP, FLAT, dg2, 2)
            # m1/m2 weights (needed from tile 16)
            splitn(wt_t, wt_d, NSLOT * 128, 3 * NSLOT * 128, dw2, 2)
            sync.dma_start(wdr_t[:, 512:1536], wdr_d[:, 512:1536]).then_inc(dw2, 16)

            # output chunks of OCH tiles; final chunk tapers into quarters
            nch = NT // OCH
            ninc = 0
            for c in range(nch):
                lo, hi = c * OCH * NFREE, (c + 1) * OCH * NFREE
                if c == nch - 1:
                    # tile 46, then tile 47 in four quarter pieces (the
                    # last two issued on sync + scalar in parallel)
                    sync.wait_ge(dve, NT - 1)
                    mid = lo + NFREE
                    sync.dma_start(out_d[:, lo:mid], st_t[:, lo:mid]).then_inc(dout, 16)
                    q = NFREE // 4
                    sync.wait_ge(dve, NT)
                    for i in range(2):
                        a = mid + i * q
                        sync.dma_start(out_d[:, a:a + q], st_t[:, a:a + q]).then_inc(dout, 16)
                    sync.wait_ge(dve, NT + 1)
                    a = mid + 2 * q
                    sync.dma_start(out_d[:, a:a + q], st_t[:, a:a + q]).then_inc(dout, 16)
                    ninc += 4
                else:
                    sync.wait_ge(dve, OCH * (c + 1))
                    sync.dma_start(out_d[:, lo:hi], st_t[:, lo:hi]).then_inc(dout, 16)
                    ninc += 1
            sync.wait_ge(dout, 16 * (ninc + 1))

        @block.tensor
        def _(tensor):
            # pre-warm the PE HAM clock gate during the initial DMA wait:
            # dummy matmuls (garbage into bank 7, which tile 7 later clears
            # with start=True) ramp the clock before the real stream begins
            # and bridge gap-free to the wave-0 DMA landing.  They must be
            # FULL-SIZE (K=128, M=128): the HAM responds to array
            # utilization, not busy-ness -- 1-partition dummies never flip
            # it.  st_t is garbage but nothing writes it until after pes
            # fires, and NaNs never leave bank 7.
            for _ in range(NDUMMY):
                nc.tensor.matmul(
                    ps_t[:, 7 * NFREE:8 * NFREE],
                    st_t[:, 0:128],
                    st_t[:, 0:NFREE],
                    start=True,
                    stop=True,
                )
            # phase A: DR+xa accumulation for tiles 0..PHA-1, gated on the
            # just-in-time x row chunks
            # tile 0 inline: bf16 slot quads as their weights land, then
            # the DR slots (gated on x8+wdr); start goes on the first bf16
            tensor.wait_ge(d0a1, 32)
            ps0 = ps_t[:, 0:NFREE]
            for i, (s, view, kh, kw) in enumerate(slots_a):
                if i == 6:
                    tensor.wait_ge(d0a2, 32)
                nc.tensor.matmul(
                    ps0, wt_ap(0, s),
                    view[:, kh: kh + RB, kw: kw + W],
                    start=(i == 0), stop=False,
                )
            tensor.wait_ge(d0b, 32)
            for pr, kw in enumerate(DR_KW):
                nc.tensor.matmul(
                    ps0, wdr_ap(0, pr),
                    x8_dr(0, kw, RB),
                    start=False, stop=False, perf_mode=DR,
                )
            tensor.wait_ge(d1a, 32)
            for k in range(1, 3):
                emit_phase_a(tensor, k, start=True)
            tensor.wait_ge(d1b, 64)
            for k in range(3, PHA):
                emit_phase_a(tensor, k, start=True)
            # phase B: finish tiles 0..PHA-1 with the xb/xc pair slots
            tensor.wait_ge(dbc, 112)
            for k in range(PHA):
                emit_phase_b(tensor, k)
            # steady state; x rows 38..51 then 52..65 arrive in two waves.
            # one bank-reuse wait covers 4 tiles: tiles k..k+3 need at most
            # dve >= k+3-(NPS-1) = k-4, and DVE lags PE by well under the
            # 3-tile slack this leaves. Fewer waits = fewer PE queue stalls.
            tensor.wait_ge(dg1, 96)
            for k in range(PHA, NT - 1):
                if k == 12:
                    tensor.wait_ge(dg2, 96)
                if k == 16:
                    tensor.wait_ge(dw2, 48)
                if (k - PHA) % 4 == 0:
                    tensor.wait_ge(dve, min(k + 3, NT - 1) - NPS + 1)
                emit_phase_a(tensor, k, start=True)
                emit_phase_b(tensor, k)
            # final tile split into two 2-row groups (N=256 in half banks):
            # the first half's epilogue+DMA overlaps the second half's
            # matmuls, shortening the kernel tail
            k = NT - 1
            m, blk = divmod(k, NBLK)
            j0 = blk * RB
            for h in range(2):
                # halves in DIFFERENT banks (7, then 6): DVE reads half 1
                # while PE accumulates half 2, and same-bank PE-write +
                # DVE-read is a fatal PSUM collision. Bank 6 (tile 46) is
                # free once dve >= NT-1.
                if h == 1:
                    tensor.wait_ge(dve, NT - 1)
                ps_h = ps_t[:, (7 - h) * NFREE:(7 - h) * NFREE + NFREE // 2]
                for pr, kw in enumerate(DR_KW):
                    nc.tensor.matmul(
                        ps_h, wdr_ap(m, pr),
                        x8_dr(j0 + 2 * h, kw, RB // 2),
                        start=(pr == 0), stop=False, perf_mode=DR,
                    )
                for sl, is_last in ((slots_a, False), (slots_bc, True)):
                    n = len(sl)
                    for i, (s, view, kh, kw) in enumerate(sl):
                        mm = nc.tensor.matmul(
                            ps_h, wt_ap(m, s),
                            view[:, j0 + 2 * h + kh: j0 + 2 * h + kh + RB // 2,
                                 kw: kw + W],
                            start=False,
                            stop=(is_last and i == n - 1),
                        )
                        if is_last and i == n - 1:
                            mm.then_inc(pes, 1)

        @block.vector
        def _(vector):
            vector.wait_ge(dvt1a, 32)  # bias + valid blks 0-1 resident
            for k in range(NT - 1):
                m, blk = divmod(k, NBLK)
                if k == 2:
                    vector.wait_ge(dvt1b, 48)
                if k == 8:
                    vector.wait_ge(dvt2, 48)
                ps = ps_t[:, (k % NPS) * NFREE:(k % NPS + 1) * NFREE]
                vector.wait_ge(pes, k + 1)
                nc.vector.scalar_tensor_tensor(
                    st_t[:, k * NFREE:(k + 1) * NFREE],
                    ps,
                    bt_t[:, m:m + 1],
                    vt_t[:, blk * NFREE:(blk + 1) * NFREE],
                    mybir.AluOpType.add,
                    mybir.AluOpType.mult,
                ).then_inc(dve, 1)
            # final tile: two half-width epilogues matching the split groups
            k = NT - 1
            m, blk = divmod(k, NBLK)
            HF = NFREE // 2
            for h in range(2):
                ps_h = ps_t[:, (7 - h) * NFREE:(7 - h) * NFREE + HF]
                vector.wait_ge(pes, k + 1 + h)
                nc.vector.scalar_tensor_tensor(
                    st_t[:, k * NFREE + h * HF:k * NFREE + (h + 1) * HF],
                    ps_h,
                    bt_t[:, m:m + 1],
                    vt_t[:, blk * NFREE + h * HF:blk * NFREE + (h + 1) * HF],
                    mybir.AluOpType.add,
                    mybir.AluOpType.mult,
                ).then_inc(dve, 1)
    return nc


def _causal_mask():
    m = np.ones((KH, KW), dtype=np.float32)
    m[KH // 2, KW // 2:] = 0.0
    m[KH // 2 + 1:, :] = 0.0
    return m


def _prepare_in_maps(x, weight, bias, mask):
    # window-any of mask -> valid [B, H, W] float32
    ind = (np.asarray(mask)[:, 0] != 0)
    indp = np.zeros((B, H + 2 * PAD, W + 2 * PAD), dtype=bool)
    indp[:, PAD:PAD + H, PAD:PAD + W] = ind
    valid = np.zeros((B, H, W), dtype=bool)
    for dh in range(KH):
        for dw in range(KW):
            valid |= indp[:, dh:dh + H, dw:dw + W]
    valid_f = valid.astype(np.float32)

    w32 = np.asarray(weight, dtype=np.float32) * _causal_mask()[None, None]
    w_bf = w32.astype(BF16)

    # bf16 weight slots, m-major: [128 ch-part, m, s, 128 cout]
    wt = np.zeros((3, NSLOT, 128, 128), dtype=BF16)
    for m in range(3):
        cs = slice(m * 128, (m + 1) * 128)
        for s, (kh, kw) in enumerate(TAPS_BF):
            wt[m, s] = w_bf[cs, 0:128, kh, kw].T
        for i, (ta, tb) in enumerate(PAIRS_XB):
            wt[m, SBC0 + i, 0:64] = w_bf[cs, 128:192, ta[0], ta[1]].T
            wt[m, SBC0 + i, 64:128] = w_bf[cs, 128:192, tb[0], tb[1]].T
        ta, tb = PAIR_XC
        wt[m, SBC0 + 5, 0:64] = w_bf[cs, 128:192, ta[0], ta[1]].T
        wt[m, SBC0 + 5, 64:128] = w_bf[cs, 128:192, tb[0], tb[1]].T
    wt_sb = np.ascontiguousarray(wt.transpose(2, 0, 1, 3)).reshape(128, -1)

    # fp8 DR weights: [128 ch, m, pair, two, 128 cout], scaled by 8
    wdr = np.zeros((3, 2, 2, 128, 128), dtype=F8)
    for m in range(3):
        cs = slice(m * 128, (m + 1) * 128)
        for pr, kw in enumerate(DR_KW):
            wdr[m, pr, 0] = (w32[cs, 0:128, 0, kw].T * 8.0).astype(F8)
            wdr[m, pr, 1] = (w32[cs, 0:128, 1, kw].T * 8.0).astype(F8)
    wdr_sb = np.ascontiguousarray(wdr.transpose(3, 0, 1, 2, 4)).reshape(128, -1)

    bias_t = np.ascontiguousarray(
        np.asarray(bias, dtype=np.float32).reshape(3, 128).T)

    x32 = np.asarray(x, dtype=np.float32)
    x_bf = x32.astype(BF16)
    x_f8 = (x32[:, 0:128] / 8.0).astype(F8)   # only ci[0:128] needed in fp8

    in_maps = []
    for c in range(NCORES):
        b, half = c // 2, c % 2
        r0 = half * HHALF
        lo = r0 - PAD
        src_lo = max(lo, 0)
        xp = np.zeros((CIN, NROWS, WP), dtype=BF16)
        xp[:, src_lo - lo:, PAD:PAD + W] = x_bf[b, :, src_lo:r0 + HHALF, :]
        xf = xp.reshape(CIN, FLAT)
        # fp8 staging: same rows, 144-col pitch, plus a +1-row-shifted copy
        xp8 = np.zeros((128, NROWS, WP8), dtype=F8)
        xp8[:, src_lo - lo:, PAD:PAD + W] = x_f8[b, :, src_lo:r0 + HHALF, :]
        x8a = xp8.reshape(128, FLAT8)
        x2 = xf[128:192]
        sh1 = np.zeros_like(x2)
        sh1[:, :-1] = x2[:, 1:]
        shr = np.zeros_like(x2)
        shr[:, :-WP] = x2[:, WP:]
        vrow = valid_f[b, r0:r0 + HHALF].reshape(1, HHALF * W).astype(BF16)
        vt = np.ascontiguousarray(np.broadcast_to(vrow, (128, HHALF * W)))
        in_maps.append({
            "xa": np.ascontiguousarray(xf[0:128]),
            "x8": x8a,
            "xb": np.ascontiguousarray(np.concatenate([x2, sh1], axis=0)),
            "xc": np.ascontiguousarray(np.concatenate([x2, shr], axis=0)),
            "wt": wt_sb,
            "wdr": wdr_sb,
            "bt": bias_t,
            "vt": vt,
        })
    return in_maps


def _assemble(results):
    out_full = np.zeros((B, COUT, H, W), dtype=np.float32)
    for c in range(NCORES):
        b, half = c // 2, c % 2
        o = np.asarray(results[c]["out"]).astype(np.float32)
        o4 = o.reshape(128, 3, HHALF, W).transpose(1, 0, 2, 3).reshape(COUT, HHALF, W)
        out_full[b, :, half * HHALF:(half + 1) * HHALF, :] = o4
    return out_full


def kernel(x, weight, bias, mask, _trace=False):
    in_maps = _prepare_in_maps(x, weight, bias, mask)
    nc = _build_program()
    res = run_bass_kernel_spmd(nc, in_maps, core_ids=list(range(NCORES)),
                               trace=_trace)
    out = _assemble(res.results)
    if _trace:
        return out, res
    return out


# revision 28
# speedup vs baseline: 1.0048x; 1.0048x over previous
"""Masked 5x5 conv (PixelCNN 'A' mask) on 8 Trainium2 NeuronCores.

Problem (hardcoded): x[4,192,128,128] f32, weight[384,192,5,5] f32,
bias[384] f32, mask[4,1,128,128] i32.
out = where(window_any(mask), conv(x, weight*maskA) + bias, 0).

The 'A' causal mask keeps 12 of 25 taps: rows kh=0,1 fully, row kh=2 only
kw=0,1 -- i.e. every tap reads the current output row or rows above it.

Sharding: core c = (batch b = c//2, row-half = c%2). Each core computes one
batch's 64 output rows for all 384 out channels (3 M=128 chunks).

Per output tile [128 cout, 4 rows x 128 cols = 512] we accumulate 16
matmuls into one PSUM bank:
  - 2 fp8 DoubleRow matmuls (K=256): tap pairs (0,0)+(1,0) and (0,2)+(1,2)
    on ci[0:128], operands e4m3 (x/8 and 8*w so products keep natural
    scale).  DoubleRow streams the two K-groups as dim-1 of a [128,2,...]
    AP; group 1 reads a row-shifted fp8 copy so the pair step is a fixed
    whole-tensor offset (16-byte aligned).
  - 8 bf16 K=128 matmuls for the remaining ci[0:128] taps (from tile xa)
  - 5 bf16 tap-PAIRS x ci[128:192]      (tile xb: lower 64 partitions =
    ci[128:192] data, upper 64 = same data shifted 1 col, so one K=128
    matmul covers two taps that differ by (0,+1))
  - 1 bf16 tap-pair (0,4)+(1,4) x ci[128:192] (tile xc: upper shifted +1 row)
4 of 18 K-slabs in fp8 keeps rel err ~0.017 (<2e-2) while cutting the PE
stream ~10%.
Epilogue: one DVE scalar_tensor_tensor: out = (psum + bias) * valid.
"""

import numpy as np
import ml_dtypes

import concourse.bass as bass
import concourse.tile as tile
from concourse import mybir
from concourse.bass_utils import run_bass_kernel_spmd

B, CIN, COUT, H, W = 4, 192, 384, 128, 128
KH = KW = 5
PAD = 2
NCORES = 8
HHALF = 64          # output rows per core
NROWS = HHALF + 2   # input rows staged per core (2 above)
WP = W + 4          # padded width (bf16 tensors)
FLAT = NROWS * WP   # 66*132 = 8712
WP8 = 144           # fp8 row pitch (16B-aligned so DoubleRow step%16==0)
FLAT8 = NROWS * WP8  # 66*144 = 9504
RB = 4              # output rows per block
NBLK = HHALF // RB  # 16 blocks
NFREE = RB * W      # 512 = one PSUM bank of fp32
NT = 3 * NBLK       # 48 tiles

# bf16 xa taps (ci 0:128) -- the 8 'A'-mask taps not covered by fp8 pairs
TAPS_BF = [(0, 1), (0, 3), (0, 4),
           (1, 1), (1, 3), (1, 4),
           (2, 0), (2, 1)]
# fp8 DoubleRow pairs: taps (0,kw)+(1,kw) on ci[0:128]
DR_KW = [0, 2]
# ci[128:192] handled as bf16 pairs packed into K=128 matmuls.
# slab xb (upper shifted +1 element = +1 col): pairs differing by (0,1)
PAIRS_XB = [((0, 0), (0, 1)), ((0, 2), (0, 3)),
            ((1, 0), (1, 1)), ((1, 2), (1, 3)), ((2, 0), (2, 1))]
# slab xc (upper shifted +132 elements = +1 row): the leftover pair
PAIR_XC = ((0, 4), (1, 4))

NSLOT = len(TAPS_BF) + len(PAIRS_XB) + 1   # 14 bf16 weight slots
SBC0 = len(TAPS_BF)                        # first xb/xc slot index

BF16 = ml_dtypes.bfloat16
F8 = ml_dtypes.float8_e4m3


def _build_program():
    """Raw Bass (no Tile): this walrus build rejects instructions carrying
    more than ~1 embedded sync wait, so all synchronization is standalone
    wait_ge instructions with manually-managed semaphores.

    Schedule (per core):
      - PE pre-warm: dummy matmuls during the initial DMA wait flip the
        HAM clock gate toward 2.4 GHz before the real stream begins.
      - Input DMAs stream in prioritized FIFO waves (queues are ~45-90
        GB/s each, ~358 GB/s aggregate); wave-1a is kept tiny (m=0
        weights for the first slots + first rows of xa/x8) so real
        matmuls start ~10us in.
      - Phase A runs the 2 DR + 8 xa slots of tiles 0..7 as the first
        x rows land; phase B completes those tiles with the xb/xc pair
        slots; then steady state: 16 matmuls per [128 x 512] PSUM tile.
      - DVE fuses (psum + bias) * valid into one scalar_tensor_tensor per
        tile, writing a bf16 staging buffer; outputs stream out in 2-tile
        chunks with a tapered, split final chunk."""
    nc = bass.Bass()
    bf = mybir.dt.bfloat16
    f8 = mybir.dt.float8e4
    f32 = mybir.dt.float32

    xa_d = nc.dram_tensor("xa", [128, FLAT], bf, kind="ExternalInput")
    x8_d = nc.dram_tensor("x8", [128, FLAT8], f8, kind="ExternalInput")
    xb_d = nc.dram_tensor("xb", [128, FLAT], bf, kind="ExternalInput")
    xc_d = nc.dram_tensor("xc", [128, FLAT], bf, kind="ExternalInput")
    wt_d = nc.dram_tensor("wt", [128, 3 * NSLOT * 128], bf, kind="ExternalInput")
    wdr_d = nc.dram_tensor("wdr", [128, 3 * 2 * 256], f8, kind="ExternalInput")
    bt_d = nc.dram_tensor("bt", [128, 3], f32, kind="ExternalInput")
    vt_d = nc.dram_tensor("vt", [128, HHALF * W], bf, kind="ExternalInput")
    out_d = nc.dram_tensor("out", [128, 3 * HHALF * W], bf, kind="ExternalOutput")

    NPS = 8           # psum banks in rotation
    PHA = 8           # tiles 0..PHA-1 run split-phase (xa/DR first, xb/xc later)
    OCH = 2           # out-DMA granularity: blocks per chunk
    NDUMMY = 13       # PE pre-warm matmuls (sized to end ~at the clock flip)
    DR = mybir.MatmulPerfMode.DoubleRow

    # row boundaries for the input waves (staged row index)
    R0 = 7            # wave 0: xa rows for tile 0
    R08 = 5           # wave 0: x8 rows for tile 0
    R1A = 14          # wave 1a: rows ..13  (tiles 1..2)
    R1B = 38          # wave 1b: rows 14..37 (tiles 3..7 + phase B)
    RG1 = 52          # wave 3 group 1: rows 38..51 (tiles 8..11)

    from contextlib import ExitStack
    with ExitStack() as ctx:
        xa_t = ctx.enter_context(nc.sbuf_tensor([128, FLAT], bf))
        x8_t = ctx.enter_context(nc.sbuf_tensor([128, FLAT8], f8))
        xb_t = ctx.enter_context(nc.sbuf_tensor([128, FLAT], bf))
        xc_t = ctx.enter_context(nc.sbuf_tensor([128, FLAT], bf))
        wt_t = ctx.enter_context(nc.sbuf_tensor([128, 3 * NSLOT * 128], bf))
        wdr_t = ctx.enter_context(nc.sbuf_tensor([128, 3 * 2 * 256], f8))
        bt_t = ctx.enter_context(nc.sbuf_tensor([128, 3], f32))
        vt_t = ctx.enter_context(nc.sbuf_tensor([128, HHALF * W], bf))
        st_t = ctx.enter_context(nc.sbuf_tensor([128, 3 * HHALF * W], bf))
        ps_t = ctx.enter_context(nc.psum_tensor([128, NPS * NFREE], f32))
        d0a1 = ctx.enter_context(nc.semaphore("d0a1"))
        d0a2 = ctx.enter_context(nc.semaphore("d0a2"))
        d0b = ctx.enter_context(nc.semaphore("d0b"))
        d1a = ctx.enter_context(nc.semaphore("d1a"))
        d1b = ctx.enter_context(nc.semaphore("d1b"))
        dbc = ctx.enter_context(nc.semaphore("dbc"))    # wt sBC m0 + xb/xc rows<38
        dvt1a = ctx.enter_context(nc.semaphore("dvt1a"))  # bt + vt blks 0-1
        dvt1b = ctx.enter_context(nc.semaphore("dvt1b"))  # vt blks 2-7
        dvt2 = ctx.enter_context(nc.semaphore("dvt2"))  # vt second half
        dg1 = ctx.enter_context(nc.semaphore("dg1"))    # x rows 38..51
        dg2 = ctx.enter_context(nc.semaphore("dg2"))    # x rows 52..65
        dw2 = ctx.enter_context(nc.semaphore("dw2"))    # wt/wdr m1,m2
        pes = ctx.enter_context(nc.semaphore("pes"))
        dve = ctx.enter_context(nc.semaphore("dve"))
        dout = ctx.enter_context(nc.semaphore("dout"))
        block = ctx.enter_context(nc.Block())
        xa_v = xa_t[:].rearrange("p (r c) -> p r c", c=WP)
        xb_v = xb_t[:].rearrange("p (r c) -> p r c", c=WP)
        xc_v = xc_t[:].rearrange("p (r c) -> p r c", c=WP)
        # DoubleRow rhs [p, two, r, c]: dim 1 (the K-group pair) strides by
        # one fp8 row (144B, 16B-aligned), overlapping the row dim -- group
        # 0 reads rows j0.. (kh=0) and group 1 rows j0+1.. (kh=1).
        from concourse.ap import AP as _AP
        _x8ap = x8_t[:]

        def x8_dr(j0, kw, rb):
            return _AP(_x8ap.tensor, j0 * WP8 + kw,
                       [[FLAT8, 128], [WP8, 2], [WP8, rb], [1, W]])

        # bf16 slots: (weight-slot index, view, kh, kw)
        slots_a = [(s, xa_v, kh, kw) for s, (kh, kw) in enumerate(TAPS_BF)]
        slots_bc = [(SBC0 + i, xb_v, ta[0], ta[1])
                    for i, (ta, _tb) in enumerate(PAIRS_XB)]
        slots_bc += [(SBC0 + 5, xc_v, PAIR_XC[0][0], PAIR_XC[0][1])]

        def wt_ap(m, s):
            o = (m * NSLOT + s) * 128
            return wt_t[:, o:o + 128]

        def wdr_ap(m, pr):
            o = (m * 2 + pr) * 256
            return wdr_t[:, o:o + 256].rearrange("p (two m1) -> p two m1", two=2)

        def emit_phase_a(tensor, k, start):
            """2 DR + 8 bf16 xa matmuls of tile k (no stop)."""
            m, blk = divmod(k, NBLK)
            j0 = blk * RB
            ps = ps_t[:, (k % NPS) * NFREE:(k % NPS + 1) * NFREE]
            for pr, kw in enumerate(DR_KW):
                nc.tensor.matmul(
                    ps, wdr_ap(m, pr),
                    x8_dr(j0, kw, RB),
                    start=(start and pr == 0), stop=False, perf_mode=DR,
                )
            for s, view, kh, kw in slots_a:
                nc.tensor.matmul(
                    ps, wt_ap(m, s),
                    view[:, j0 + kh: j0 + kh + RB, kw: kw + W],
                    start=False, stop=False,
                )

        def emit_phase_b(tensor, k):
            """6 bf16 xb/xc pair matmuls of tile k (stop on last)."""
            m, blk = divmod(k, NBLK)
            j0 = blk * RB
            ps = ps_t[:, (k % NPS) * NFREE:(k % NPS + 1) * NFREE]
            n = len(slots_bc)
            for i, (s, view, kh, kw) in enumerate(slots_bc):
                mm = nc.tensor.matmul(
                    ps, wt_ap(m, s),
                    view[:, j0 + kh: j0 + kh + RB, kw: kw + W],
                    start=False, stop=(i == n - 1),
                )
                if i == n - 1:
                    mm.then_inc(pes, 1)

        @block.scalar
        def _(scalar):
            # the very last output quarter, issued in parallel with sync's
            scalar.wait_ge(dve, NT + 1)
            a = (NT - 1) * NFREE + 3 * (NFREE // 4)
            scalar.dma_start(out_d[:, a:a + NFREE // 4],
                             st_t[:, a:a + NFREE // 4]).then_inc(dout, 16)

        @block.sync
        def _(sync):
            # Queues drain FIFO, so pure issue order gives wave priority.
            # Each dma_start costs ~650ns of issue time on this engine, so
            # the head waves use few, need-ordered descriptors.
            def splitn(dst, src, lo, hi, sem, n):
                step = ((hi - lo) // n // 16) * 16
                for i in range(n):
                    a = lo + i * step
                    b = hi if i == n - 1 else a + step
                    sync.dma_start(dst[:, a:b], src[:, a:b]).then_inc(sem, 16)

            # wave 0: exactly what tile 0 needs, gated at slot-group
            # granularity so its matmuls start as pieces land
            splitn(xa_t, xa_d, 0, R0 * WP, d0a1, 1)
            splitn(wt_t, wt_d, 0, 4 * 128, d0a1, 1)
            splitn(wt_t, wt_d, 4 * 128, SBC0 * 128, d0a2, 1)
            splitn(x8_t, x8_d, 0, R08 * WP8, d0b, 1)
            sync.dma_start(wdr_t[:, 0:512], wdr_d[:, 0:512]).then_inc(d0b, 16)
            # wave 1a: x rows out to 13 (tiles 1..2)
            splitn(xa_t, xa_d, R0 * WP, R1A * WP, d1a, 1)
            splitn(x8_t, x8_d, R08 * WP8, R1A * WP8, d1a, 1)
            # wave 1b: x rows 14..37 for tiles 3..7  (6 DMAs)
            splitn(xa_t, xa_d, R1A * WP, R1B * WP, d1b, 2)
            splitn(x8_t, x8_d, R1A * WP8, R1B * WP8, d1b, 2)
            # wave 2: phase-B inputs first, then the valid/bias epilogue
            # inputs (vt in a small leading chunk so the first stt can run)
            sync.dma_start(bt_t[:], bt_d[:]).then_inc(dvt1a, 16)
            sync.dma_start(wt_t[:, SBC0 * 128:NSLOT * 128],
                           wt_d[:, SBC0 * 128:NSLOT * 128]).then_inc(dbc, 16)
            splitn(xb_t, xb_d, 0, R1B * WP, dbc, 3)
            splitn(xc_t, xc_d, 0, R1B * WP, dbc, 3)
            splitn(vt_t, vt_d, 0, 2 * RB * W, dvt1a, 1)
            splitn(vt_t, vt_d, 2 * RB * W, (HHALF // 2) * W, dvt1b, 3)
            # wave 3 group 1: all x, rows 38..51 (tiles 8..11)
            splitn(xa_t, xa_d, R1B * WP, RG1 * WP, dg1, 2)
            splitn(x8_t, x8_d, R1B * WP8, RG1 * WP8, dg1, 1)
            splitn(xb_t, xb_d, R1B * WP, RG1 * WP, dg1, 2)
            splitn(xc_t, xc_d, R1B * WP, RG1 * WP, dg1, 2)
            # vt second half (DVE needs it from tile 8)
            splitn(vt_t, vt_d, (HHALF // 2) * W, HHALF * W, dvt2, 3)
            # wave 3 group 2: all x, rows 52..65 (tiles 12..15)
            splitn(xa_t, xa_d, RG1 * WP, FLAT, dg2, 2)
            splitn(x8_t, x8_d, RG1 * WP8, FLAT8, dg2, 1)
            splitn(xb_t, xb_d, RG1 * WP, FLAT, dg2, 2)
            splitn(xc_t, xc_d, RG1 * WP, FLAT, dg2, 2)
            # m1/m2 weights (needed from tile 16)
            splitn(wt_t, wt_d, NSLOT * 128, 3 * NSLOT * 128, dw2, 2)
            sync.dma_start(wdr_t[:, 512:1536], wdr_d[:, 512:1536]).then_inc(dw2, 16)

            # output chunks of OCH tiles; final chunk tapers into quarters
            nch = NT // OCH
            ninc = 0
            for c in range(nch):
                lo, hi = c * OCH * NFREE, (c + 1) * OCH * NFREE
                if c == nch - 1:
                    # tile 46, then tile 47 in four quarter pieces (the
                    # last two issued on sync + scalar in parallel)
                    sync.wait_ge(dve, NT - 1)
                    mid = lo + NFREE
                    sync.dma_start(out_d[:, lo:mid], st_t[:, lo:mid]).then_inc(dout, 16)
                    q = NFREE // 4
                    sync.wait_ge(dve, NT)
                    for i in range(2):
                        a = mid + i * q
                        sync.dma_start(out_d[:, a:a + q], st_t[:, a:a + q]).then_inc(dout, 16)
                    sync.wait_ge(dve, NT + 1)
                    a = mid + 2 * q
                    sync.dma_start(out_d[:, a:a + q], st_t[:, a:a + q]).then_inc(dout, 16)
                    ninc += 4
                else:
                    sync.wait_ge(dve, OCH * (c + 1))
                    sync.dma_start(out_d[:, lo:hi], st_t[:, lo:hi]).then_inc(dout, 16)
                    ninc += 1
            sync.wait_ge(dout, 16 * (ninc + 1))

        @block.tensor
        def _(tensor):
            # pre-warm the PE HAM clock gate during the initial DMA wait:
            # dummy matmuls (garbage into bank 7, which tile 7 later clears
            # with start=True) ramp the clock before the real stream begins
            # and bridge gap-free to the wave-0 DMA landing.  They must be
            # FULL-SIZE (K=128, M=128): the HAM responds to array
            # utilization, not busy-ness -- 1-partition dummies never flip
            # it.  st_t is garbage but nothing writes it until after pes
            # fires, and NaNs never leave bank 7.
            for _ in range(NDUMMY):
                nc.tensor.matmul(
                    ps_t[:, 7 * NFREE:8 * NFREE],
                    st_t[:, 0:128],
                    st_t[:, 0:NFREE],
                    start=True,
                    stop=True,
                )
            # phase A: DR+xa accumulation for tiles 0..PHA-1, gated on the
            # just-in-time x row chunks
            # tile 0 inline: bf16 slot quads as their weights land, then
            # the DR slots (gated on x8+wdr); start goes on the first bf16
            tensor.wait_ge(d0a1, 32)
            ps0 = ps_t[:, 0:NFREE]
            for i, (s, view, kh, kw) in enumerate(slots_a):
                if i == 4:
                    tensor.wait_ge(d0a2, 16)
                nc.tensor.matmul(
                    ps0, wt_ap(0, s),
                    view[:, kh: kh + RB, kw: kw + W],
                    start=(i == 0), stop=False,
                )
            tensor.wait_ge(d0b, 32)
            for pr, kw in enumerate(DR_KW):
                nc.tensor.matmul(
                    ps0, wdr_ap(0, pr),
                    x8_dr(0, kw, RB),
                    start=False, stop=False, perf_mode=DR,
                )
            tensor.wait_ge(d1a, 32)
            for k in range(1, 3):
                emit_phase_a(tensor, k, start=True)
            tensor.wait_ge(d1b, 64)
            for k in range(3, PHA):
                emit_phase_a(tensor, k, start=True)
            # phase B: finish tiles 0..PHA-1 with the xb/xc pair slots
            tensor.wait_ge(dbc, 112)
            for k in range(PHA):
                emit_phase_b(tensor, k)
            # steady state; x rows 38..51 then 52..65 arrive in two waves.
            # one bank-reuse wait covers 4 tiles: tiles k..k+3 need at most
            # dve >= k+3-(NPS-1) = k-4, and DVE lags PE by well under the
            # 3-tile slack this leaves. Fewer waits = fewer PE queue stalls.
            tensor.wait_ge(dg1, 96)
            for k in range(PHA, NT - 1):
                if k == 12:
                    tensor.wait_ge(dg2, 96)
                if k == 16:
                    tensor.wait_ge(dw2, 48)
                if (k - PHA) % 4 == 0:
                    tensor.wait_ge(dve, min(k + 3, NT - 1) - NPS + 1)
                emit_phase_a(tensor, k, start=True)
                emit_phase_b(tensor, k)
            # final tile split into two 2-row groups (N=256 in half banks):
            # the first half's epilogue+DMA overlaps the second half's
            # matmuls, shortening the kernel tail
            k = NT - 1
            m, blk = divmod(k, NBLK)
            j0 = blk * RB
            for h in range(2):
                # halves in DIFFERENT banks (7, then 6): DVE reads half 1
                # while PE accumulates half 2, and same-bank PE-write +
                # DVE-read is a fatal PSUM collision. Bank 6 (tile 46) is
                # free once dve >= NT-1.
                if h == 1:
                    tensor.wait_ge(dve, NT - 1)
                ps_h = ps_t[:, (7 - h) * NFREE:(7 - h) * NFREE + NFREE // 2]
                for pr, kw in enumerate(DR_KW):
                    nc.tensor.matmul(
                        ps_h, wdr_ap(m, pr),
                        x8_dr(j0 + 2 * h, kw, RB // 2),
                        start=(pr == 0), stop=False, perf_mode=DR,
                    )
                for sl, is_last in ((slots_a, False), (slots_bc, True)):
                    n = len(sl)
                    for i, (s, view, kh, kw) in enumerate(sl):
                        mm = nc.tensor.matmul(
                            ps_h, wt_ap(m, s),
                            view[:, j0 + 2 * h + kh: j0 + 2 * h + kh + RB // 2,
                                 kw: kw + W],
                            start=False,
                            stop=(is_last and i == n - 1),
                        )
                        if is_last and i == n - 1:
                            mm.then_inc(pes, 1)

        @block.vector
        def _(vector):
            vector.wait_ge(dvt1a, 32)  # bias + valid blks 0-1 resident
            for k in range(NT - 1):
                m, blk = divmod(k, NBLK)
                if k == 2:
                    vector.wait_ge(dvt1b, 48)
                if k == 8:
                    vector.wait_ge(dvt2, 48)
                ps = ps_t[:, (k % NPS) * NFREE:(k % NPS + 1) * NFREE]
                vector.wait_ge(pes, k + 1)
                nc.vector.scalar_tensor_tensor(
                    st_t[:, k * NFREE:(k + 1) * NFREE],
                    ps,
                    bt_t[:, m:m + 1],
                    vt_t[:, blk * NFREE:(blk + 1) * NFREE],
                    mybir.AluOpType.add,
                    mybir.AluOpType.mult,
                ).then_inc(dve, 1)
            # final tile: two half-width epilogues matching the split groups
            k = NT - 1
            m, blk = divmod(k, NBLK)
            HF = NFREE // 2
            for h in range(2):
                ps_h = ps_t[:, (7 - h) * NFREE:(7 - h) * NFREE + HF]
                vector.wait_ge(pes, k + 1 + h)
                nc.vector.scalar_tensor_tensor(
                    st_t[:, k * NFREE + h * HF:k * NFREE + (h + 1) * HF],
                    ps_h,
                    bt_t[:, m:m + 1],
                    vt_t[:, blk * NFREE + h * HF:blk * NFREE + (h + 1) * HF],
                    mybir.AluOpType.add,
                    mybir.AluOpType.mult,
                ).then_inc(dve, 1)
    return nc


def _causal_mask():
    m = np.ones((KH, KW), dtype=np.float32)
    m[KH // 2, KW // 2:] = 0.0
    m[KH // 2 + 1:, :] = 0.0
    return m


def _prepare_in_maps(x, weight, bias, mask):
    # window-any of mask -> valid [B, H, W] float32
    ind = (np.asarray(mask)[:, 0] != 0)
    indp = np.zeros((B, H + 2 * PAD, W + 2 * PAD), dtype=bool)
    indp[:, PAD:PAD + H, PAD:PAD + W] = ind
    valid = np.zeros((B, H, W), dtype=bool)
    for dh in range(KH):
        for dw in range(KW):
            valid |= indp[:, dh:dh + H, dw:dw + W]
    valid_f = valid.astype(np.float32)

    w32 = np.asarray(weight, dtype=np.float32) * _causal_mask()[None, None]
    w_bf = w32.astype(BF16)

    # bf16 weight slots, m-major: [128 ch-part, m, s, 128 cout]
    wt = np.zeros((3, NSLOT, 128, 128), dtype=BF16)
    for m in range(3):
        cs = slice(m * 128, (m + 1) * 128)
        for s, (kh, kw) in enumerate(TAPS_BF):
            wt[m, s] = w_bf[cs, 0:128, kh, kw].T
        for i, (ta, tb) in enumerate(PAIRS_XB):
            wt[m, SBC0 + i, 0:64] = w_bf[cs, 128:192, ta[0], ta[1]].T
            wt[m, SBC0 + i, 64:128] = w_bf[cs, 128:192, tb[0], tb[1]].T
        ta, tb = PAIR_XC
        wt[m, SBC0 + 5, 0:64] = w_bf[cs, 128:192, ta[0], ta[1]].T
        wt[m, SBC0 + 5, 64:128] = w_bf[cs, 128:192, tb[0], tb[1]].T
    wt_sb = np.ascontiguousarray(wt.transpose(2, 0, 1, 3)).reshape(128, -1)

    # fp8 DR weights: [128 ch, m, pair, two, 128 cout], scaled by 8
    wdr = np.zeros((3, 2, 2, 128, 128), dtype=F8)
    for m in range(3):
        cs = slice(m * 128, (m + 1) * 128)
        for pr, kw in enumerate(DR_KW):
            wdr[m, pr, 0] = (w32[cs, 0:128, 0, kw].T * 8.0).astype(F8)
            wdr[m, pr, 1] = (w32[cs, 0:128, 1, kw].T * 8.0).astype(F8)
    wdr_sb = np.ascontiguousarray(wdr.transpose(3, 0, 1, 2, 4)).reshape(128, -1)

    bias_t = np.ascontiguousarray(
        np.asarray(bias, dtype=np.float32).reshape(3, 128).T)

    x32 = np.asarray(x, dtype=np.float32)
    x_bf = x32.astype(BF16)
    x_f8 = (x32[:, 0:128] / 8.0).astype(F8)   # only ci[0:128] needed in fp8

    in_maps = []
    for c in range(NCORES):
        b, half = c // 2, c % 2
        r0 = half * HHALF
        lo = r0 - PAD
        src_lo = max(lo, 0)
        xp = np.zeros((CIN, NROWS, WP), dtype=BF16)
        xp[:, src_lo - lo:, PAD:PAD + W] = x_bf[b, :, src_lo:r0 + HHALF, :]
        xf = xp.reshape(CIN, FLAT)
        # fp8 staging: same rows, 144-col pitch, plus a +1-row-shifted copy
        xp8 = np.zeros((128, NROWS, WP8), dtype=F8)
        xp8[:, src_lo - lo:, PAD:PAD + W] = x_f8[b, :, src_lo:r0 + HHALF, :]
        x8a = xp8.reshape(128, FLAT8)
        x2 = xf[128:192]
        sh1 = np.zeros_like(x2)
        sh1[:, :-1] = x2[:, 1:]
        shr = np.zeros_like(x2)
        shr[:, :-WP] = x2[:, WP:]
        vrow = valid_f[b, r0:r0 + HHALF].reshape(1, HHALF * W).astype(BF16)
        vt = np.ascontiguousarray(np.broadcast_to(vrow, (128, HHALF * W)))
        in_maps.append({
            "xa": np.ascontiguousarray(xf[0:128]),
            "x8": x8a,
            "xb": np.ascontiguousarray(np.concatenate([x2, sh1], axis=0)),
            "xc": np.ascontiguousarray(np.concatenate([x2, shr], axis=0)),
            "wt": wt_sb,
            "wdr": wdr_sb,
            "bt": bias_t,
            "vt": vt,
        })
    return in_maps


def _assemble(results):
    out_full = np.zeros((B, COUT, H, W), dtype=np.float32)
    for c in range(NCORES):
        b, half = c // 2, c % 2
        o = np.asarray(results[c]["out"]).astype(np.float32)
        o4 = o.reshape(128, 3, HHALF, W).transpose(1, 0, 2, 3).reshape(COUT, HHALF, W)
        out_full[b, :, half * HHALF:(half + 1) * HHALF, :] = o4
    return out_full


def kernel(x, weight, bias, mask, _trace=False):
    in_maps = _prepare_in_maps(x, weight, bias, mask)
    nc = _build_program()
    res = run_bass_kernel_spmd(nc, in_maps, core_ids=list(range(NCORES)),
                               trace=_trace)
    out = _assemble(res.results)
    if _trace:
        return out, res
    return out


# revision 29
# speedup vs baseline: 1.0125x; 1.0076x over previous
"""Masked 5x5 conv (PixelCNN 'A' mask) on 8 Trainium2 NeuronCores.

Problem (hardcoded): x[4,192,128,128] f32, weight[384,192,5,5] f32,
bias[384] f32, mask[4,1,128,128] i32.
out = where(window_any(mask), conv(x, weight*maskA) + bias, 0).

The 'A' causal mask keeps 12 of 25 taps: rows kh=0,1 fully, row kh=2 only
kw=0,1 -- i.e. every tap reads the current output row or rows above it.

Sharding: core c = (batch b = c//2, row-half = c%2). Each core computes one
batch's 64 output rows for all 384 out channels (3 M=128 chunks).

Per output tile [128 cout, 4 rows x 128 cols = 512] we accumulate 16
matmuls into one PSUM bank:
  - 2 fp8 DoubleRow matmuls (K=256): tap pairs (0,0)+(1,0) and (0,2)+(1,2)
    on ci[0:128], operands e4m3 (x/8 and 8*w so products keep natural
    scale).  DoubleRow streams the two K-groups as dim-1 of a [128,2,...]
    AP; group 1 reads a row-shifted fp8 copy so the pair step is a fixed
    whole-tensor offset (16-byte aligned).
  - 8 bf16 K=128 matmuls for the remaining ci[0:128] taps (from tile xa)
  - 5 bf16 tap-PAIRS x ci[128:192]      (tile xb: lower 64 partitions =
    ci[128:192] data, upper 64 = same data shifted 1 col, so one K=128
    matmul covers two taps that differ by (0,+1))
  - 1 bf16 tap-pair (0,4)+(1,4) x ci[128:192] (tile xc: upper shifted +1 row)
4 of 18 K-slabs in fp8 keeps rel err ~0.017 (<2e-2) while cutting the PE
stream ~10%.
Epilogue: one DVE scalar_tensor_tensor: out = (psum + bias) * valid.
"""

import numpy as np
import ml_dtypes

import concourse.bass as bass
import concourse.tile as tile
from concourse import mybir
from concourse.bass_utils import run_bass_kernel_spmd

B, CIN, COUT, H, W = 4, 192, 384, 128, 128
KH = KW = 5
PAD = 2
NCORES = 8
HHALF = 64          # output rows per core
NROWS = HHALF + 2   # input rows staged per core (2 above)
WP = W + 4          # padded width (bf16 tensors)
FLAT = NROWS * WP   # 66*132 = 8712
WP8 = 144           # fp8 row pitch (16B-aligned so DoubleRow step%16==0)
FLAT8 = NROWS * WP8  # 66*144 = 9504
RB = 4              # output rows per block
NBLK = HHALF // RB  # 16 blocks
NFREE = RB * W      # 512 = one PSUM bank of fp32
NT = 3 * NBLK       # 48 tiles

# bf16 xa taps (ci 0:128) -- the 8 'A'-mask taps not covered by fp8 pairs
TAPS_BF = [(0, 1), (0, 3), (0, 4),
           (1, 1), (1, 3), (1, 4),
           (2, 0), (2, 1)]
# fp8 DoubleRow pairs: taps (0,kw)+(1,kw) on ci[0:128]
DR_KW = [0, 2]
# ci[128:192] handled as bf16 pairs packed into K=128 matmuls.
# slab xb (upper shifted +1 element = +1 col): pairs differing by (0,1)
PAIRS_XB = [((0, 0), (0, 1)), ((0, 2), (0, 3)),
            ((1, 0), (1, 1)), ((1, 2), (1, 3)), ((2, 0), (2, 1))]
# slab xc (upper shifted +132 elements = +1 row): the leftover pair
PAIR_XC = ((0, 4), (1, 4))

NSLOT = len(TAPS_BF) + len(PAIRS_XB) + 1   # 14 bf16 weight slots
SBC0 = len(TAPS_BF)                        # first xb/xc slot index

BF16 = ml_dtypes.bfloat16
F8 = ml_dtypes.float8_e4m3


def _build_program():
    """Raw Bass (no Tile): this walrus build rejects instructions carrying
    more than ~1 embedded sync wait, so all synchronization is standalone
    wait_ge instructions with manually-managed semaphores.

    Schedule (per core):
      - PE pre-warm: dummy matmuls during the initial DMA wait flip the
        HAM clock gate toward 2.4 GHz before the real stream begins.
      - Input DMAs stream in prioritized FIFO waves (queues are ~45-90
        GB/s each, ~358 GB/s aggregate); wave-1a is kept tiny (m=0
        weights for the first slots + first rows of xa/x8) so real
        matmuls start ~10us in.
      - Phase A runs the 2 DR + 8 xa slots of tiles 0..7 as the first
        x rows land; phase B completes those tiles with the xb/xc pair
        slots; then steady state: 16 matmuls per [128 x 512] PSUM tile.
      - DVE fuses (psum + bias) * valid into one scalar_tensor_tensor per
        tile, writing a bf16 staging buffer; outputs stream out in 2-tile
        chunks with a tapered, split final chunk."""
    nc = bass.Bass()
    bf = mybir.dt.bfloat16
    f8 = mybir.dt.float8e4
    f32 = mybir.dt.float32

    xa_d = nc.dram_tensor("xa", [128, FLAT], bf, kind="ExternalInput")
    x8_d = nc.dram_tensor("x8", [128, FLAT8], f8, kind="ExternalInput")
    xb_d = nc.dram_tensor("xb", [128, FLAT], bf, kind="ExternalInput")
    xc_d = nc.dram_tensor("xc", [128, FLAT], bf, kind="ExternalInput")
    wt_d = nc.dram_tensor("wt", [128, 3 * NSLOT * 128], bf, kind="ExternalInput")
    wdr_d = nc.dram_tensor("wdr", [128, 3 * 2 * 256], f8, kind="ExternalInput")
    bt_d = nc.dram_tensor("bt", [128, 3], f32, kind="ExternalInput")
    vt_d = nc.dram_tensor("vt", [128, HHALF * W], bf, kind="ExternalInput")
    out_d = nc.dram_tensor("out", [128, 3 * HHALF * W], bf, kind="ExternalOutput")

    NPS = 8           # psum banks in rotation
    PHA = 8           # tiles 0..PHA-1 run split-phase (xa/DR first, xb/xc later)
    OCH = 2           # out-DMA granularity: blocks per chunk
    NDUMMY = 13       # PE pre-warm matmuls (sized to end ~at the clock flip)
    DR = mybir.MatmulPerfMode.DoubleRow

    # row boundaries for the input waves (staged row index)
    R0 = 7            # wave 0: xa rows for tile 0
    R08 = 5           # wave 0: x8 rows for tile 0
    R1A = 14          # wave 1a: rows ..13  (tiles 1..2)
    R1B = 38          # wave 1b: rows 14..37 (tiles 3..7 + phase B)
    RG1 = 52          # wave 3 group 1: rows 38..51 (tiles 8..11)

    from contextlib import ExitStack
    with ExitStack() as ctx:
        xa_t = ctx.enter_context(nc.sbuf_tensor([128, FLAT], bf))
        x8_t = ctx.enter_context(nc.sbuf_tensor([128, FLAT8], f8))
        xb_t = ctx.enter_context(nc.sbuf_tensor([128, FLAT], bf))
        xc_t = ctx.enter_context(nc.sbuf_tensor([128, FLAT], bf))
        wt_t = ctx.enter_context(nc.sbuf_tensor([128, 3 * NSLOT * 128], bf))
        wdr_t = ctx.enter_context(nc.sbuf_tensor([128, 3 * 2 * 256], f8))
        bt_t = ctx.enter_context(nc.sbuf_tensor([128, 3], f32))
        vt_t = ctx.enter_context(nc.sbuf_tensor([128, HHALF * W], bf))
        st_t = ctx.enter_context(nc.sbuf_tensor([128, 3 * HHALF * W], bf))
        ps_t = ctx.enter_context(nc.psum_tensor([128, NPS * NFREE], f32))
        d0a1 = ctx.enter_context(nc.semaphore("d0a1"))
        d0a2 = ctx.enter_context(nc.semaphore("d0a2"))
        d0b = ctx.enter_context(nc.semaphore("d0b"))
        d1a = ctx.enter_context(nc.semaphore("d1a"))
        d1b = ctx.enter_context(nc.semaphore("d1b"))
        dbc = ctx.enter_context(nc.semaphore("dbc"))    # wt sBC m0 + xb/xc rows<38
        dvt1a = ctx.enter_context(nc.semaphore("dvt1a"))  # bt + vt blks 0-1
        dvt1b = ctx.enter_context(nc.semaphore("dvt1b"))  # vt blks 2-7
        dvt2 = ctx.enter_context(nc.semaphore("dvt2"))  # vt second half
        dg1 = ctx.enter_context(nc.semaphore("dg1"))    # x rows 38..51
        dg2 = ctx.enter_context(nc.semaphore("dg2"))    # x rows 52..65
        dw2 = ctx.enter_context(nc.semaphore("dw2"))    # wt/wdr m1,m2
        pes = ctx.enter_context(nc.semaphore("pes"))
        dve = ctx.enter_context(nc.semaphore("dve"))
        dout = ctx.enter_context(nc.semaphore("dout"))
        block = ctx.enter_context(nc.Block())
        xa_v = xa_t[:].rearrange("p (r c) -> p r c", c=WP)
        xb_v = xb_t[:].rearrange("p (r c) -> p r c", c=WP)
        xc_v = xc_t[:].rearrange("p (r c) -> p r c", c=WP)
        # DoubleRow rhs [p, two, r, c]: dim 1 (the K-group pair) strides by
        # one fp8 row (144B, 16B-aligned), overlapping the row dim -- group
        # 0 reads rows j0.. (kh=0) and group 1 rows j0+1.. (kh=1).
        from concourse.ap import AP as _AP
        _x8ap = x8_t[:]

        def x8_dr(j0, kw, rb):
            return _AP(_x8ap.tensor, j0 * WP8 + kw,
                       [[FLAT8, 128], [WP8, 2], [WP8, rb], [1, W]])

        # bf16 slots: (weight-slot index, view, kh, kw)
        slots_a = [(s, xa_v, kh, kw) for s, (kh, kw) in enumerate(TAPS_BF)]
        slots_bc = [(SBC0 + i, xb_v, ta[0], ta[1])
                    for i, (ta, _tb) in enumerate(PAIRS_XB)]
        slots_bc += [(SBC0 + 5, xc_v, PAIR_XC[0][0], PAIR_XC[0][1])]

        def wt_ap(m, s):
            o = (m * NSLOT + s) * 128
            return wt_t[:, o:o + 128]

        def wdr_ap(m, pr):
            o = (m * 2 + pr) * 256
            return wdr_t[:, o:o + 256].rearrange("p (two m1) -> p two m1", two=2)

        def emit_phase_a(tensor, k, start):
            """2 DR + 8 bf16 xa matmuls of tile k (no stop)."""
            m, blk = divmod(k, NBLK)
            j0 = blk * RB
            ps = ps_t[:, (k % NPS) * NFREE:(k % NPS + 1) * NFREE]
            for pr, kw in enumerate(DR_KW):
                nc.tensor.matmul(
                    ps, wdr_ap(m, pr),
                    x8_dr(j0, kw, RB),
                    start=(start and pr == 0), stop=False, perf_mode=DR,
                )
            for s, view, kh, kw in slots_a:
                nc.tensor.matmul(
                    ps, wt_ap(m, s),
                    view[:, j0 + kh: j0 + kh + RB, kw: kw + W],
                    start=False, stop=False,
                )

        def emit_phase_b(tensor, k):
            """6 bf16 xb/xc pair matmuls of tile k (stop on last)."""
            m, blk = divmod(k, NBLK)
            j0 = blk * RB
            ps = ps_t[:, (k % NPS) * NFREE:(k % NPS + 1) * NFREE]
            n = len(slots_bc)
            for i, (s, view, kh, kw) in enumerate(slots_bc):
                mm = nc.tensor.matmul(
                    ps, wt_ap(m, s),
                    view[:, j0 + kh: j0 + kh + RB, kw: kw + W],
                    start=False, stop=(i == n - 1),
                )
                if i == n - 1:
                    mm.then_inc(pes, 1)

        @block.scalar
        def _(scalar):
            # the very last output quarter, issued in parallel with sync's
            scalar.wait_ge(dve, NT + 2)
            a = (NT - 1) * NFREE + 3 * (NFREE // 4)
            scalar.dma_start(out_d[:, a:a + NFREE // 4],
                             st_t[:, a:a + NFREE // 4]).then_inc(dout, 16)

        @block.sync
        def _(sync):
            # Queues drain FIFO, so pure issue order gives wave priority.
            # Each dma_start costs ~650ns of issue time on this engine, so
            # the head waves use few, need-ordered descriptors.
            def splitn(dst, src, lo, hi, sem, n):
                step = ((hi - lo) // n // 16) * 16
                for i in range(n):
                    a = lo + i * step
                    b = hi if i == n - 1 else a + step
                    sync.dma_start(dst[:, a:b], src[:, a:b]).then_inc(sem, 16)

            # wave 0: exactly what tile 0 needs, gated at slot-group
            # granularity so its matmuls start as pieces land
            splitn(xa_t, xa_d, 0, R0 * WP, d0a1, 1)
            splitn(wt_t, wt_d, 0, 4 * 128, d0a1, 1)
            splitn(wt_t, wt_d, 4 * 128, SBC0 * 128, d0a2, 1)
            splitn(x8_t, x8_d, 0, R08 * WP8, d0b, 1)
            sync.dma_start(wdr_t[:, 0:512], wdr_d[:, 0:512]).then_inc(d0b, 16)
            # wave 1a: x rows out to 13 (tiles 1..2)
            splitn(xa_t, xa_d, R0 * WP, R1A * WP, d1a, 1)
            splitn(x8_t, x8_d, R08 * WP8, R1A * WP8, d1a, 1)
            # wave 1b: x rows 14..37 for tiles 3..7  (6 DMAs)
            splitn(xa_t, xa_d, R1A * WP, R1B * WP, d1b, 2)
            splitn(x8_t, x8_d, R1A * WP8, R1B * WP8, d1b, 2)
            # wave 2: phase-B inputs first, then the valid/bias epilogue
            # inputs (vt in a small leading chunk so the first stt can run)
            sync.dma_start(bt_t[:], bt_d[:]).then_inc(dvt1a, 16)
            sync.dma_start(wt_t[:, SBC0 * 128:NSLOT * 128],
                           wt_d[:, SBC0 * 128:NSLOT * 128]).then_inc(dbc, 16)
            splitn(xb_t, xb_d, 0, R1B * WP, dbc, 3)
            splitn(xc_t, xc_d, 0, R1B * WP, dbc, 3)
            splitn(vt_t, vt_d, 0, 2 * RB * W, dvt1a, 1)
            splitn(vt_t, vt_d, 2 * RB * W, (HHALF // 2) * W, dvt1b, 3)
            # wave 3 group 1: all x, rows 38..51 (tiles 8..11)
            splitn(xa_t, xa_d, R1B * WP, RG1 * WP, dg1, 2)
            splitn(x8_t, x8_d, R1B * WP8, RG1 * WP8, dg1, 1)
            splitn(xb_t, xb_d, R1B * WP, RG1 * WP, dg1, 2)
            splitn(xc_t, xc_d, R1B * WP, RG1 * WP, dg1, 2)
            # vt second half (DVE needs it from tile 8)
            splitn(vt_t, vt_d, (HHALF // 2) * W, HHALF * W, dvt2, 3)
            # wave 3 group 2: all x, rows 52..65 (tiles 12..15)
            splitn(xa_t, xa_d, RG1 * WP, FLAT, dg2, 2)
            splitn(x8_t, x8_d, RG1 * WP8, FLAT8, dg2, 1)
            splitn(xb_t, xb_d, RG1 * WP, FLAT, dg2, 2)
            splitn(xc_t, xc_d, RG1 * WP, FLAT, dg2, 2)
            # m1/m2 weights (needed from tile 16)
            splitn(wt_t, wt_d, NSLOT * 128, 3 * NSLOT * 128, dw2, 2)
            sync.dma_start(wdr_t[:, 512:1536], wdr_d[:, 512:1536]).then_inc(dw2, 16)

            # output chunks of OCH tiles; final chunk tapers into quarters
            nch = NT // OCH
            ninc = 0
            for c in range(nch):
                lo, hi = c * OCH * NFREE, (c + 1) * OCH * NFREE
                if c == nch - 1:
                    # tile 46, then tile 47 in four quarter pieces (the
                    # last two issued on sync + scalar in parallel)
                    sync.wait_ge(dve, NT - 1)
                    mid = lo + NFREE
                    sync.dma_start(out_d[:, lo:mid], st_t[:, lo:mid]).then_inc(dout, 16)
                    q = NFREE // 4
                    sync.wait_ge(dve, NT)
                    for i in range(2):
                        a = mid + i * q
                        sync.dma_start(out_d[:, a:a + q], st_t[:, a:a + q]).then_inc(dout, 16)
                    sync.wait_ge(dve, NT + 1)
                    a = mid + 2 * q
                    sync.dma_start(out_d[:, a:a + q], st_t[:, a:a + q]).then_inc(dout, 16)
                    ninc += 4
                    # (the last quarter is issued by the scalar engine)
                else:
                    sync.wait_ge(dve, OCH * (c + 1))
                    sync.dma_start(out_d[:, lo:hi], st_t[:, lo:hi]).then_inc(dout, 16)
                    ninc += 1
            sync.wait_ge(dout, 16 * (ninc + 1))

        @block.tensor
        def _(tensor):
            # pre-warm the PE HAM clock gate during the initial DMA wait:
            # dummy matmuls (garbage into bank 7, which tile 7 later clears
            # with start=True) ramp the clock before the real stream begins
            # and bridge gap-free to the wave-0 DMA landing.  They must be
            # FULL-SIZE (K=128, M=128): the HAM responds to array
            # utilization, not busy-ness -- 1-partition dummies never flip
            # it.  st_t is garbage but nothing writes it until after pes
            # fires, and NaNs never leave bank 7.
            for _ in range(NDUMMY):
                nc.tensor.matmul(
                    ps_t[:, 7 * NFREE:8 * NFREE],
                    st_t[:, 0:128],
                    st_t[:, 0:NFREE],
                    start=True,
                    stop=True,
                )
            # phase A: DR+xa accumulation for tiles 0..PHA-1, gated on the
            # just-in-time x row chunks
            # tile 0 inline: bf16 slot quads as their weights land, then
            # the DR slots (gated on x8+wdr); start goes on the first bf16
            tensor.wait_ge(d0a1, 32)
            ps0 = ps_t[:, 0:NFREE]
            for i, (s, view, kh, kw) in enumerate(slots_a):
                if i == 4:
                    tensor.wait_ge(d0a2, 16)
                nc.tensor.matmul(
                    ps0, wt_ap(0, s),
                    view[:, kh: kh + RB, kw: kw + W],
                    start=(i == 0), stop=False,
                )
            tensor.wait_ge(d0b, 32)
            for pr, kw in enumerate(DR_KW):
                nc.tensor.matmul(
                    ps0, wdr_ap(0, pr),
                    x8_dr(0, kw, RB),
                    start=False, stop=False, perf_mode=DR,
                )
            tensor.wait_ge(d1a, 32)
            for k in range(1, 3):
                emit_phase_a(tensor, k, start=True)
            tensor.wait_ge(d1b, 64)
            for k in range(3, PHA):
                emit_phase_a(tensor, k, start=True)
            # phase B: finish tiles 0..PHA-1 with the xb/xc pair slots
            tensor.wait_ge(dbc, 112)
            for k in range(PHA):
                emit_phase_b(tensor, k)
            # steady state; x rows 38..51 then 52..65 arrive in two waves.
            # one bank-reuse wait covers 4 tiles: tiles k..k+3 need at most
            # dve >= k+3-(NPS-1) = k-4, and DVE lags PE by well under the
            # 3-tile slack this leaves. Fewer waits = fewer PE queue stalls.
            tensor.wait_ge(dg1, 96)
            for k in range(PHA, NT - 1):
                if k == 12:
                    tensor.wait_ge(dg2, 96)
                if k == 16:
                    tensor.wait_ge(dw2, 48)
                if (k - PHA) % 4 == 0:
                    tensor.wait_ge(dve, min(k + 3, NT - 1) - NPS + 1)
                emit_phase_a(tensor, k, start=True)
                emit_phase_b(tensor, k)
            # final tile split into two 2-row groups (N=256 in half banks):
            # the first half's epilogue+DMA overlaps the second half's
            # matmuls, shortening the kernel tail
            k = NT - 1
            m, blk = divmod(k, NBLK)
            j0 = blk * RB
            for h in range(2):
                # halves in DIFFERENT banks (7, then 6): DVE reads half 1
                # while PE accumulates half 2, and same-bank PE-write +
                # DVE-read is a fatal PSUM collision. Bank 6 (tile 46) is
                # free once dve >= NT-1.
                if h == 1:
                    tensor.wait_ge(dve, NT - 1)
                ps_h = ps_t[:, (7 - h) * NFREE:(7 - h) * NFREE + NFREE // 2]
                for pr, kw in enumerate(DR_KW):
                    nc.tensor.matmul(
                        ps_h, wdr_ap(m, pr),
                        x8_dr(j0 + 2 * h, kw, RB // 2),
                        start=(pr == 0), stop=False, perf_mode=DR,
                    )
                for sl, is_last in ((slots_a, False), (slots_bc, True)):
                    n = len(sl)
                    for i, (s, view, kh, kw) in enumerate(sl):
                        mm = nc.tensor.matmul(
                            ps_h, wt_ap(m, s),
                            view[:, j0 + 2 * h + kh: j0 + 2 * h + kh + RB // 2,
                                 kw: kw + W],
                            start=False,
                            stop=(is_last and i == n - 1),
                        )
                        if is_last and i == n - 1:
                            mm.then_inc(pes, 1)

        @block.vector
        def _(vector):
            vector.wait_ge(dvt1a, 32)  # bias + valid blks 0-1 resident
            for k in range(NT - 1):
                m, blk = divmod(k, NBLK)
                if k == 2:
                    vector.wait_ge(dvt1b, 48)
                if k == 8:
                    vector.wait_ge(dvt2, 48)
                ps = ps_t[:, (k % NPS) * NFREE:(k % NPS + 1) * NFREE]
                vector.wait_ge(pes, k + 1)
                nc.vector.scalar_tensor_tensor(
                    st_t[:, k * NFREE:(k + 1) * NFREE],
                    ps,
                    bt_t[:, m:m + 1],
                    vt_t[:, blk * NFREE:(blk + 1) * NFREE],
                    mybir.AluOpType.add,
                    mybir.AluOpType.mult,
                ).then_inc(dve, 1)
            # final tile: half-width epilogue for group 0, then two
            # quarter-width ones for group 1 so the last out-DMAs chain
            # off the earliest possible dve increments
            k = NT - 1
            m, blk = divmod(k, NBLK)
            HF = NFREE // 2
            QF = NFREE // 4
            pieces = [(0, 0, HF), (1, 0, QF), (1, QF, 2 * QF)]
            for h, a, b in pieces:
                ps_h = ps_t[:, (7 - h) * NFREE:(7 - h) * NFREE + HF]
                vector.wait_ge(pes, k + 1 + h)
                nc.vector.scalar_tensor_tensor(
                    st_t[:, k * NFREE + h * HF + a:k * NFREE + h * HF + b],
                    ps_h[:, a:b],
                    bt_t[:, m:m + 1],
                    vt_t[:, blk * NFREE + h * HF + a:blk * NFREE + h * HF + b],
                    mybir.AluOpType.add,
                    mybir.AluOpType.mult,
                ).then_inc(dve, 1)
    return nc


def _causal_mask():
    m = np.ones((KH, KW), dtype=np.float32)
    m[KH // 2, KW // 2:] = 0.0
    m[KH // 2 + 1:, :] = 0.0
    return m


def _prepare_in_maps(x, weight, bias, mask):
    # window-any of mask -> valid [B, H, W] float32
    ind = (np.asarray(mask)[:, 0] != 0)
    indp = np.zeros((B, H + 2 * PAD, W + 2 * PAD), dtype=bool)
    indp[:, PAD:PAD + H, PAD:PAD + W] = ind
    valid = np.zeros((B, H, W), dtype=bool)
    for dh in range(KH):
        for dw in range(KW):
            valid |= indp[:, dh:dh + H, dw:dw + W]
    valid_f = valid.astype(np.float32)

    w32 = np.asarray(weight, dtype=np.float32) * _causal_mask()[None, None]
    w_bf = w32.astype(BF16)

    # bf16 weight slots, m-major: [128 ch-part, m, s, 128 cout]
    wt = np.zeros((3, NSLOT, 128, 128), dtype=BF16)
    for m in range(3):
        cs = slice(m * 128, (m + 1) * 128)
        for s, (kh, kw) in enumerate(TAPS_BF):
            wt[m, s] = w_bf[cs, 0:128, kh, kw].T
        for i, (ta, tb) in enumerate(PAIRS_XB):
            wt[m, SBC0 + i, 0:64] = w_bf[cs, 128:192, ta[0], ta[1]].T
            wt[m, SBC0 + i, 64:128] = w_bf[cs, 128:192, tb[0], tb[1]].T
        ta, tb = PAIR_XC
        wt[m, SBC0 + 5, 0:64] = w_bf[cs, 128:192, ta[0], ta[1]].T
        wt[m, SBC0 + 5, 64:128] = w_bf[cs, 128:192, tb[0], tb[1]].T
    wt_sb = np.ascontiguousarray(wt.transpose(2, 0, 1, 3)).reshape(128, -1)

    # fp8 DR weights: [128 ch, m, pair, two, 128 cout], scaled by 8
    wdr = np.zeros((3, 2, 2, 128, 128), dtype=F8)
    for m in range(3):
        cs = slice(m * 128, (m + 1) * 128)
        for pr, kw in enumerate(DR_KW):
            wdr[m, pr, 0] = (w32[cs, 0:128, 0, kw].T * 8.0).astype(F8)
            wdr[m, pr, 1] = (w32[cs, 0:128, 1, kw].T * 8.0).astype(F8)
    wdr_sb = np.ascontiguousarray(wdr.transpose(3, 0, 1, 2, 4)).reshape(128, -1)

    bias_t = np.ascontiguousarray(
        np.asarray(bias, dtype=np.float32).reshape(3, 128).T)

    x32 = np.asarray(x, dtype=np.float32)
    x_bf = x32.astype(BF16)
    x_f8 = (x32[:, 0:128] / 8.0).astype(F8)   # only ci[0:128] needed in fp8

    in_maps = []
    for c in range(NCORES):
        b, half = c // 2, c % 2
        r0 = half * HHALF
        lo = r0 - PAD
        src_lo = max(lo, 0)
        xp = np.zeros((CIN, NROWS, WP), dtype=BF16)
        xp[:, src_lo - lo:, PAD:PAD + W] = x_bf[b, :, src_lo:r0 + HHALF, :]
        xf = xp.reshape(CIN, FLAT)
        # fp8 staging: same rows, 144-col pitch, plus a +1-row-shifted copy
        xp8 = np.zeros((128, NROWS, WP8), dtype=F8)
        xp8[:, src_lo - lo:, PAD:PAD + W] = x_f8[b, :, src_lo:r0 + HHALF, :]
        x8a = xp8.reshape(128, FLAT8)
        x2 = xf[128:192]
        sh1 = np.zeros_like(x2)
        sh1[:, :-1] = x2[:, 1:]
        shr = np.zeros_like(x2)
        shr[:, :-WP] = x2[:, WP:]
        vrow = valid_f[b, r0:r0 + HHALF].reshape(1, HHALF * W).astype(BF16)
        vt = np.ascontiguousarray(np.broadcast_to(vrow, (128, HHALF * W)))
        in_maps.append({
            "xa": np.ascontiguousarray(xf[0:128]),
            "x8": x8a,
            "xb": np.ascontiguousarray(np.concatenate([x2, sh1], axis=0)),
            "xc": np.ascontiguousarray(np.concatenate([x2, shr], axis=0)),
            "wt": wt_sb,
            "wdr": wdr_sb,
            "bt": bias_t,
            "vt": vt,
        })
    return in_maps


def _assemble(results):
    out_full = np.zeros((B, COUT, H, W), dtype=np.float32)
    for c in range(NCORES):
        b, half = c // 2, c % 2
        o = np.asarray(results[c]["out"]).astype(np.float32)
        o4 = o.reshape(128, 3, HHALF, W).transpose(1, 0, 2, 3).reshape(COUT, HHALF, W)
        out_full[b, :, half * HHALF:(half + 1) * HHALF, :] = o4
    return out_full


def kernel(x, weight, bias, mask, _trace=False):
    in_maps = _prepare_in_maps(x, weight, bias, mask)
    nc = _build_program()
    res = run_bass_kernel_spmd(nc, in_maps, core_ids=list(range(NCORES)),
                               trace=_trace)
    out = _assemble(res.results)
    if _trace:
        return out, res
    return out
